# revision 16
# baseline (speedup 1.0000x reference)
"""Trainium2 Bass kernel for nn_MelPCENPreprocessor (v3: symmetric DFT).

Pipeline: audio (N,32000) -> reflect-pad -> STFT(400/160, hann) power
-> mel(128) -> PCEN (IIR smooth + pointwise) -> bilinear resize (201->192)
-> (N,1,192,128).

v3 mapping (vs v2's 16-matmul f32r DFT):
  * DFT uses the k <-> 400-k symmetry of the hann-windowed real DFT:
      Xc[f] = sum_{k=1..199} w[k]cos(2pi k f/400) u[k] + 0.5(-1)^f u[200]
      Xs[f] = sum_{k=1..199} w[k]sin(2pi k f/400) v[k]
    with u[k]=x[k]+x[400-k], v[k]=x[k]-x[400-k] staged on host in fp16.
    8 matmuls per sample-pair instead of 16; fp16 rounding ~2^-11.
  * power: cos-chunk squares on ACT (Square), sin.. wait f-chunk1 [128p]
    squares on ACT, f-chunk2 [71p] on DVE (self tensor_mul); adds on Pool
    (bf16, SBUF). mel matmul bf16.
  * PCEN scan on DVE reads mel PSUM directly (bf16 out, fp32 carry);
    pointwise Ln/Exp batched [128,4,202] strided views at sample stride
    256 so one DMA-transpose per quad ([128,1024] -> [128,8,128] blocks)
    replaces the 8 PE transposes + PSUM evac copies.
  * resize reads transpose blocks straight from SBUF; -sqrt(2) is folded
    into the rz PSUM evacuation (resize rows sum to 1).

Per core: N/8 samples, pure data parallel, no collectives.
"""
import numpy as np
import ml_dtypes

import concourse.bass as bass
import concourse.bacc as bacc
import concourse.mybir as mybir
from concourse import tile
from concourse.bass_utils import run_bass_kernel_spmd

SR = 16000
N_FFT = 400
HOP = 160
N_MELS = 128
F_MAX = 8000.0
S = 0.04
ALPHA = 0.8
DELTA = 2.0
FLOOR = 1e-08
T = 201           # frames per sample
TT = 192          # resized time
PAD = 200
CPS = 202         # staged cols per sample (201 frames + 1 pad)
NW = 404          # moving dim per 2-sample pair
SST = 256         # sample stride in tail tiles (transpose block multiple)
KB = 100          # k-rows per uv partition block
F32 = mybir.dt.float32
BF16 = mybir.dt.bfloat16
FP16 = mybir.dt.float16
BF16NP = ml_dtypes.bfloat16
SQRT2 = float(np.sqrt(2.0))
K0 = 4            # leading mel columns computed exactly on host

# uv blocks: 0 = u k 1..100, 1 = u k 101..200, 2 = v k 1..100,
# 3 = v k 101..199 (99 rows). W chunks follow the same order.
WROWS = [100, 100, 100, 99]
FC = [(0, 128), (128, 71)]   # freq col chunks (f-1 offsets into 199)


# ---------------- constant matrices (host, fp64 -> fp16/bf16) -------------

def _hann():
    n = np.arange(N_FFT)
    return 0.5 * (1.0 - np.cos(2.0 * np.pi * n / N_FFT))


def _mel_fb():
    n_freqs = N_FFT // 2 + 1
    all_freqs = np.linspace(0.0, SR / 2, n_freqs)

    def h2m(f):
        return 2595.0 * np.log10(1.0 + f / 700.0)

    m_pts = np.linspace(h2m(0.0), h2m(F_MAX), N_MELS + 2)
    f_pts = 700.0 * (10.0 ** (m_pts / 2595.0) - 1.0)
    f_diff = f_pts[1:] - f_pts[:-1]
    slopes = f_pts[None, :] - all_freqs[:, None]
    down = -slopes[:, :-2] / f_diff[:-1]
    up = slopes[:, 2:] / f_diff[1:]
    return np.maximum(0.0, np.minimum(down, up))  # (201,128) f64


def _wsym():
    """Symmetric DFT weights: Wc (200,199) incl. the k=200 row as
    0.5*(-1)^f (staged u[200] = 2*x[200]), Ws (199,199); f = 1..199."""
    w = _hann()
    k = np.arange(1, 200)[:, None]
    f = np.arange(1, 200)[None, :]
    ang = 2.0 * np.pi * k * f / N_FFT
    wc = np.concatenate(
        [w[1:200, None] * np.cos(ang),
         0.5 * ((-1.0) ** f)], axis=0)          # (200, 199)
    ws = w[1:200, None] * np.sin(ang)           # (199, 199)
    return wc, ws


def _resize_r():
    scale = TT / T
    sample_f = (np.arange(TT, dtype=np.float64) + 0.5) / scale - 0.5
    j = np.arange(T, dtype=np.float64)[None, :]
    w = np.maximum(0.0, 1.0 - np.abs((j - sample_f[:, None]) * scale))
    w = w / w.sum(axis=1, keepdims=True)
    return w  # (192, 201), rows sum to 1


def _consts():
    wc, ws = _wsym()
    # wsym [100, 4*199] fp16: chunk c covers k-rows per WROWS
    wsym = np.zeros((KB, 4 * 199), np.float64)
    wsym[0:100, 0:199] = wc[0:100]
    wsym[0:100, 199:398] = wc[100:200]
    wsym[0:100, 398:597] = ws[0:100]
    wsym[0:99, 597:796] = ws[100:199]
    fb = _mel_fb()[1:200]                       # (199,128)
    rt = _resize_r().T                          # (201,192)
    ball = np.zeros((128, 128 + 128 + 192 + 192), np.float64)
    ball[0:128, 0:128] = fb[0:128]
    ball[0:71, 128:256] = fb[128:199]
    ball[0:128, 256:448] = rt[0:128]
    ball[0:73, 448:640] = rt[128:201]
    ballq = np.ascontiguousarray(ball).astype(BF16NP)
    # -sqrt(2) folded after resize: subtract sqrt2 * (bf16 row sums of RT)
    rs = (ballq[:, 256:448].astype(np.float64).sum(axis=0)
          + ballq[0:73, 448:640].astype(np.float64).sum(axis=0)) * SQRT2
    rsc = np.zeros((128, 2), np.float32)
    rsc[0:128, 0] = rs[0:128]
    rsc[0:64, 1] = rs[128:192]
    return {"wsym": wsym.astype(np.float16), "ball": ballq, "rsc": rsc}


CONST_DT = {"wsym": FP16, "ball": BF16, "rsc": F32}
CONST_SHAPES = {"wsym": (KB, 4 * 199), "ball": (128, 640), "rsc": (128, 2)}


# ---------------- host input staging ----------------

def _stage(audio):
    """audio (N,32000) f32 -> fp16 u/v blocks + exact leading mel cols."""
    N = audio.shape[0]
    nquad = N // 4
    xp = np.pad(audio, ((0, 0), (PAD, PAD)), mode="reflect")  # (N, 32400)
    # col 201 (the pad frame) makes the k-views reach up to 32559 -> pad
    xpp = np.pad(xp, ((0, 0), (0, 160)), mode="constant")
    s0, s1 = xpp.strides
    # v1[n,i,j] = xp[n, 1 + i + 160j]  (k = 1 + i)
    v1 = np.lib.stride_tricks.as_strided(
        xpp[:, 1:], shape=(N, 200, CPS), strides=(s0, s1, s1 * HOP),
        writeable=False)
    # v2[n,i,j] = xp[n, 399 - i + 160j]  (400 - k)
    v2 = np.lib.stride_tricks.as_strided(
        xpp[:, 399:], shape=(N, 200, CPS), strides=(s0, -s1, s1 * HOP),
        writeable=False)
    u = (v1 + v2).astype(np.float16)            # (N, 200, 202) k=1..200
    v = (v1[:, 0:199] - v2[:, 0:199]).astype(np.float16)

    uv = np.zeros((N, 4, KB, CPS), np.float16)
    uv[:, 0] = u[:, 0:100]
    uv[:, 1] = u[:, 100:200]
    uv[:, 2] = v[:, 0:100]
    uv[:, 3, 0:99] = v[:, 100:199]
    # -> [nquad, KB, 4 blocks, 4 samples, CPS] -> [nquad, KB, 3232]
    uvq = np.ascontiguousarray(
        uv.reshape(nquad, 4, 4, KB, CPS).transpose(0, 3, 2, 1, 4)
        .reshape(nquad, KB, 4 * 4 * CPS))

    # exact mel for frames 0..K0-1 (f64 host math), quad-major bf16
    w = _hann()
    kk = np.arange(N_FFT)[:, None]
    ff = np.arange(1, 200)[None, :]
    aa = 2.0 * np.pi * kk * ff / N_FFT
    wcf = w[:, None] * np.cos(aa)
    wsf = w[:, None] * np.sin(aa)
    fb = _mel_fb()[1:200]
    fr = np.stack([xp[:, HOP * j:HOP * j + N_FFT] for j in range(K0)],
                  axis=1).astype(np.float64)    # (N, K0, 400)
    melf = ((fr @ wcf) ** 2 + (fr @ wsf) ** 2) @ fb   # (N, K0, 128)
    mf = np.ascontiguousarray(
        melf.reshape(nquad, 4, K0, 128).transpose(0, 3, 1, 2)
        .reshape(nquad, 128, 4 * K0)).astype(BF16NP)
    return {"UV": uvq, "MF": mf}


# ---------------- device program ----------------

def emit_quad(nc, csb, c96, floor_c, delta_c, din, dout, pools, q):
    """Loads + DFT + power + mel + scan for one quad. Returns tail state."""
    (xpool, wpool, opool, ps_dft, ps_mel, ps_rz) = pools
    A = mybir.ActivationFunctionType

    uv = xpool.tile([KB, 16 * CPS], FP16, tag="UV", name="uv")
    mf = xpool.tile([128, 4 * K0], BF16, tag="MF", name="mf")
    nc.sync.dma_start(uv[:, :], din["UV"][q])
    nc.sync.dma_start(mf[:, :], din["MF"][q])
    uvv = uv[:, :].rearrange("p (b c) -> p b c", b=4)

    mels = [ps_mel.tile([128, NW], F32, tag="mel", name=f"mel{p}", bufs=2)
            for p in range(2)]
    mp = wpool.tile([128, 4 * SST], BF16, tag="mp", name="mp")
    init = wpool.tile([128, 4], F32, tag="init", name="init")

    for p in range(2):
        x0 = NW * p
        # ---- DFT: cos into bank0, sin into bank1; 2 f-chunks ----
        d1 = ps_dft.tile([128, 1024], F32, tag="d1", name="d1")
        d2 = ps_dft.tile([71, 1024], F32, tag="d2", name="d2")
        for ti, base in ((0, 0), (1, 2)):     # 0: cos (blocks 0,1), 1: sin
            for fi, (f0, fw) in enumerate(FC):
                dst = (d1 if fi == 0 else d2)[:, :].rearrange(
                    "p (b u) -> p b u", b=2)[0:fw, ti, 0:NW]
                for kc in range(2):
                    c = base + kc
                    rows = WROWS[c]
                    nc.tensor.matmul(
                        dst,
                        csb["wsym"][0:rows, 199 * c + f0:199 * c + f0 + fw],
                        uvv[0:rows, c, x0:x0 + NW],
                        start=(kc == 0), stop=(kc == 1))

        # ---- power ----
        # f-chunk1 (and pair-1 chunk2): ACT Square straight off PSUM;
        # pair-0 chunk2: DVE copy -> bf16 self-mul (TT cannot read two
        # PSUM operands). Adds on Pool / DVE to balance.
        sq1 = wpool.tile([128, 2 * NW], BF16, tag="sq1", name="sq1")
        sq2 = wpool.tile([71, 2 * NW], BF16, tag="sq2", name="sq2")
        nc.scalar.activation(
            sq1[:, :].rearrange("p (b u) -> p b u", b=2),
            d1[:, :].rearrange("p (b u) -> p b u", b=2)[:, :, 0:NW],
            A.Square)
        d2v = d2[:, :].rearrange("p (b u) -> p b u", b=2)[:, :, 0:NW]
        s2v = sq2[:, :].rearrange("p (b u) -> p b u", b=2)
        pw1 = wpool.tile([128, NW], BF16, tag="pw1", name="pw1")
        pw2 = wpool.tile([71, NW], BF16, tag="pw2", name="pw2")
        if p == 0:
            x2 = wpool.tile([71, 2 * NW], BF16, tag="x2", name="x2")
            x2v = x2[:, :].rearrange("p (b u) -> p b u", b=2)
            nc.vector.tensor_copy(x2v, d2v)
            nc.vector.tensor_mul(s2v, x2v, x2v)
            nc.vector.tensor_add(pw2[:, :], sq2[:, 0:NW], sq2[:, NW:2 * NW])
        else:
            nc.scalar.activation(s2v, d2v, A.Square)
            nc.gpsimd.tensor_add(pw2[:, :], sq2[:, 0:NW],
                                 sq2[:, NW:2 * NW])
        nc.gpsimd.tensor_add(pw1[:, :], sq1[:, 0:NW], sq1[:, NW:2 * NW])

        # ---- mel (bf16) ----
        mel = mels[p]
        nc.tensor.matmul(mel[:, :], csb["ball"][0:128, 0:128], pw1[:, :],
                         start=True, stop=False)
        nc.tensor.matmul(mel[:, :], csb["ball"][0:71, 128:256], pw2[:, :],
                         start=False, stop=True)

        # exact leading mel columns for both samples (one strided copy)
        nc.vector.tensor_copy(
            mel[:, :].rearrange("p (h c) -> p h c", h=2)[:, :, 0:K0],
            mf[:, :].rearrange("p (h c) -> p h c", h=4)[:, 2 * p:2 * p + 2, :])

        # ---- PCEN scan (DVE, mel read from PSUM, bf16 out, f32 carry) ----
        nc.vector.tensor_scalar_mul(
            init[:, :].rearrange("p (h c) -> p h c", h=4)[:, 2 * p:2 * p + 2, :],
            mel[:, :].rearrange("p (h c) -> p h c", h=2)[:, :, 0:1],
            1.0 / S)
        for h in range(2):
            s = 2 * p + h
            nc.vector.tensor_tensor_scan(
                mp[:, SST * s:SST * s + CPS], c96[:, 0:CPS],
                mel[:, CPS * h:CPS * h + CPS], init[:, s:s + 1],
                mybir.AluOpType.mult, mybir.AluOpType.add)

    return (mels, mp, q)


def emit_tail(nc, csb, c96, floor_c, delta_c, dout, pools, state, warm):
    """Pointwise tail + transpose/resize/store for a previously emitted
    quad (deferred one iteration to keep ACT off the critical path)."""
    (xpool, wpool, opool, ps_dft, ps_mel, ps_rz) = pools
    A = mybir.ActivationFunctionType
    (mels, mp, q) = state

    t1 = wpool.tile([128, 4 * SST], F32, tag="t1", name="t1")
    t2 = wpool.tile([128, 4 * SST], BF16, tag="t2", name="t2")
    t2m = wpool.tile([128, 4 * SST], BF16, tag="t2m", name="t2m")
    t4f = wpool.tile([128, 4 * SST], F32, tag="t4f", name="t4f")
    t4 = wpool.tile([128, 4 * SST], BF16, tag="t4", name="t4")
    if not warm:
        # one-time: keep the never-written stride-gap cols finite so the
        # full-tile DMA transpose only ever moves finite bits
        nc.vector.memset(t4[:, :], 0.0)

    def sview(t, w=CPS):
        return t[:, :].rearrange("p (s c) -> p s c", s=4)[:, :, 0:w]

    # t1 = ln(S*mp + floor) ; t2 = (M+floor)^-alpha
    nc.scalar.activation(sview(t1), sview(mp), A.Ln,
                         bias=floor_c[:, 0:1], scale=S)
    nc.scalar.activation(sview(t2), sview(t1), A.Exp, scale=-ALPHA)
    # t2m = mel * t2  (mel still in PSUM; DVE — Pool cannot read PSUM)
    for p in range(2):
        mel = mels[p]
        nc.vector.tensor_mul(
            t2m[:, :].rearrange("p (pp s c) -> p pp s c", pp=2, s=2)
            [:, p, :, 0:CPS],
            mel[:, :].rearrange("p (h c) -> p h c", h=2),
            t2[:, :].rearrange("p (pp s c) -> p pp s c", pp=2, s=2)
            [:, p, :, 0:CPS])
    # t4 = sqrt(t2m + delta) via ln/exp(0.5) (same ACT table set); the
    # -sqrt(2) must land BEFORE the bf16 downcast (values >= 0 keep the
    # rounding relative; subtracting after bf16 cancels near sqrt(2))
    nc.scalar.activation(sview(t1), sview(t2m), A.Ln, bias=delta_c[:, 0:1])
    nc.scalar.activation(sview(t4f), sview(t1), A.Exp, scale=0.5)
    nc.vector.tensor_scalar_sub(sview(t4), sview(t4f), SQRT2)

    # ---- one DMA transpose: [128, 1024] -> blocks [128, 8, 128] ----
    tr = wpool.tile([128, 8 * 128], BF16, tag="tr", name="tr")
    trv = tr[:, :].rearrange("p (b c) -> p b c", b=8)
    nc.sync.dma_start_transpose(trv, t4[:, :])

    # ---- resize: 4 matmuls per pair on transpose blocks ----
    rz = ps_rz.tile([128, 1024], F32, tag="rz", name="rz")
    trb = tr[:, :].rearrange("p (s b c) -> p s b c", s=4, b=2)
    for p in range(2):
        m0 = trb[0:128, 2 * p:2 * p + 2, 0, :]      # time 0..127, 2 samples
        m1 = trb[0:73, 2 * p:2 * p + 2, 1, :]       # time 128..200
        for gi, (r0, rows) in enumerate(((0, 128), (128, 64))):
            out = rz[0:rows, 512 * p + 256 * gi:512 * p + 256 * gi + 256]
            nc.tensor.matmul(out,
                             csb["ball"][0:128, 256 + r0:256 + r0 + rows],
                             m0, start=True, stop=False)
            nc.tensor.matmul(out,
                             csb["ball"][0:73, 448 + r0:448 + r0 + rows],
                             m1, start=False, stop=True)

    # ---- evac (+ -sqrt2 fold) and store ----
    o1 = opool.tile([128, 512], F32, tag="o1", name="o1")
    o2 = opool.tile([64, 512], F32, tag="o2", name="o2")
    rzv = rz[:, :].rearrange("p (pp g c) -> p pp g c", pp=2, g=2)
    nc.vector.tensor_copy(
        o1[:, :].rearrange("p (pp c) -> p pp c", pp=2),
        rzv[0:128, :, 0, :])
    nc.vector.tensor_copy(
        o2[:, :].rearrange("p (pp c) -> p pp c", pp=2),
        rzv[0:64, :, 1, :])
    nc.sync.dma_start(dout["o1"][q], o1[:, :])
    nc.sync.dma_start(dout["o2"][q], o2[:, :])


def _build_program(nper):
    assert nper % 4 == 0
    nquad = nper // 4
    nc = bacc.Bacc("TRN2", target_bir_lowering=False, debug=False,
                   num_devices=1)

    din = {"UV": nc.dram_tensor("UV", [nquad, KB, 16 * CPS], FP16,
                                kind="ExternalInput"),
           "MF": nc.dram_tensor("MF", [nquad, 128, 4 * K0], BF16,
                                kind="ExternalInput")}
    dc = {k: nc.dram_tensor(k, list(CONST_SHAPES[k]), CONST_DT[k],
                            kind="ExternalInput")
          for k in CONST_SHAPES}
    dout = {"o1": nc.dram_tensor("o1", [nquad, 128, 512], F32,
                                 kind="ExternalOutput"),
            "o2": nc.dram_tensor("o2", [nquad, 64, 512], F32,
                                 kind="ExternalOutput")}

    with tile.TileContext(nc) as tc:
        with (
            tc.tile_pool(name="const", bufs=1) as cpool,
            tc.tile_pool(name="xin", bufs=3) as xpool,
            tc.tile_pool(name="work", bufs=3) as wpool,
            tc.tile_pool(name="outs", bufs=3) as opool,
            tc.tile_pool(name="ps_dft", bufs=1, space="PSUM") as ps_dft,
            tc.tile_pool(name="ps_mel", bufs=1, space="PSUM") as ps_mel,
            tc.tile_pool(name="ps_rz", bufs=1, space="PSUM") as ps_rz,
        ):
            cbase = {}
            for k, shp in CONST_SHAPES.items():
                t = cpool.tile(list(shp), CONST_DT[k], tag=k, name=f"c_{k}")
                nc.sync.dma_start(t[:, :], dc[k][:, :])
                cbase[k] = t
            csb = {"wsym": cbase["wsym"], "ball": cbase["ball"],
                   "rsc": cbase["rsc"]}
            c96 = cpool.tile([128, CPS], FP16, tag="c96")
            nc.vector.memset(c96[:, :], 1.0 - S)
            floor_c = cpool.tile([128, 1], F32, tag="floor_c")
            nc.vector.memset(floor_c[:, :], FLOOR)
            delta_c = cpool.tile([128, 1], F32, tag="delta_c")
            nc.vector.memset(delta_c[:, :], DELTA)

            pools = (xpool, wpool, opool, ps_dft, ps_mel, ps_rz)
            tail_state = None
            tails = 0
            for q in range(nquad):
                st = emit_quad(nc, csb, c96, floor_c, delta_c, din, dout,
                               pools, q)
                if tail_state is not None:
                    emit_tail(nc, csb, c96, floor_c, delta_c, dout,
                              pools, tail_state, warm=(tails >= 3))
                    tails += 1
                tail_state = st
            emit_tail(nc, csb, c96, floor_c, delta_c, dout, pools,
                      tail_state, warm=(tails >= 3))

    nc.finalize()
    _dedupe_act_loads(nc)
    return nc


def _dedupe_act_loads(nc):
    """Square/Ln/Exp live in one table set; drop redundant reloads."""
    from concourse.hw_specs import get_activation_tables
    import concourse.mybir as _mb
    A = _mb.ActivationFunctionType
    tables = get_activation_tables(nc.m.arch)
    set_id = None
    for i, (name, s) in enumerate(tables.items()):
        if {A.Square, A.Ln, A.Exp} <= s:
            set_id = i
            break
    assert set_id is not None
    for blk in nc.m.functions[0].blocks:
        keep = []
        seen = False
        for inst in blk.instructions:
            if type(inst).__name__ == "InstLoadActFuncSet":
                si = inst.sync_info
                if si is not None and (si.on_wait or si.on_update):
                    inst.act_func_set_id = set_id
                    keep.append(inst)
                    seen = True
                elif not seen:
                    inst.act_func_set_id = set_id
                    keep.append(inst)
                    seen = True
            else:
                keep.append(inst)
        blk.instructions[:] = keep
    return nc


_CACHE = {}


def _program(nper):
    if nper not in _CACHE:
        _CACHE[nper] = _build_program(nper)
    return _CACHE[nper]


def kernel(audio):
    audio = np.ascontiguousarray(np.asarray(audio, dtype=np.float32))
    n_orig = audio.shape[0]
    if n_orig % 4 != 0:
        pad = 4 - n_orig % 4
        audio = np.concatenate(
            [audio, np.zeros((pad, audio.shape[1]), np.float32)])
    N = audio.shape[0]
    n_cores = 8 if N % 32 == 0 else 1
    nper = N // n_cores
    nq = nper // 4
    staged = _stage(audio)
    consts = _consts()
    nc = _program(nper)
    in_maps = []
    for c in range(n_cores):
        sl = slice(c * nq, (c + 1) * nq)
        m = {k: v[sl] for k, v in staged.items()}
        m.update(consts)
        in_maps.append(m)
    res = run_bass_kernel_spmd(nc, in_maps, list(range(n_cores))).results
    o1 = np.concatenate([res[c]["o1"] for c in range(n_cores)], axis=0)
    o2 = np.concatenate([res[c]["o2"] for c in range(n_cores)], axis=0)
    # (nq, t, 4*128) -> (N, t, 128)
    top = o1.reshape(N // 4, 128, 4, 128).transpose(0, 2, 1, 3)
    bot = o2.reshape(N // 4, 64, 4, 128).transpose(0, 2, 1, 3)
    out = np.concatenate([top, bot], axis=2).reshape(N, TT, 128)
    return np.ascontiguousarray(out[:n_orig]).reshape(n_orig, 1, TT, 128)


if __name__ == "__main__":
    a = np.random.randn(32, 32000).astype(np.float32)
    o = kernel(a)
    print("kernel ok", o.shape, o.dtype, float(o.min()), float(o.max()))
